# revision 5
# baseline (speedup 1.0000x reference)
"""Trainium2 Bass kernel for GQA attention prefill (Llama-style).

Reference computation (fp32):
  xq = x@wq.T+bq; xk = x@wk.T+bk; xv = x@wv.T+bv
  rope(xq, xk); scores = q@k.T/sqrt(128) + causal_mask
  probs = softmax(scores); out = (probs@v) reshaped @ wo.T + bo

Shapes: x [2, 2048, 4096], 32 q heads / 8 kv heads, head_dim 128.

Sharding: TP=8 over head groups — core c owns q heads 4c..4c+3 and kv head c
(GQA group = exactly one core), both batches. Each core computes a partial
[2*2048, 4096] output (its heads' contribution through its wo columns); the
host sums the 8 partials per batch. bq/bk applied on device; bv/bo folded
into a constant host-side row (sum_k probs == 1).

Single fused pass per core, software-pipelined over 8 token chunks
(batch, 512-token qc). Per chunk: QKV projection (+rope) -> attention for
the 4 heads -> output projection of those 512 rows. The next chunk's
projection matmuls are interleaved between attention heads so the PE never
waits on the exp (ACT) engine, and weights (w, wo) stay SBUF-resident the
whole kernel. x is streamed via a host-side relayout that gives 8 KB
contiguous lines per partition (full DMA efficiency).

Layouts on device (all matmuls bf16, fp32 PSUM):
  - projections produce q/k/v in [head_dim, tok]; rope via de-interleaved
    head_dim (host permutes wq/wk rows: evens then odds), rotation = swap of
    64-partition halves via SBUF-SBUF DMA, sign folded into sin.
  - scores transposed S_T[k, q]; softmax denominator = DVE-accumulated
    exp-sum reduced by one ones-column matmul per (head, chunk).
  - causal masking: column-trimmed score blocks + one [128,128] triangular
    binary multiply per diagonal block.
  - PV output lands in [head_dim, tok] == lhsT of the wo matmul.
"""
import sys

for _p in ("/opt/trn_rl_repo",):
    if _p not in sys.path:
        sys.path.insert(0, _p)

from contextlib import ExitStack

import ml_dtypes
import numpy as np

import concourse.bass as bass  # noqa: F401  (AP types used implicitly)
import concourse.tile as tile
from concourse import bacc, mybir
from concourse import masks as masks_mod
from concourse.bass_utils import run_bass_kernel_spmd

BF16 = ml_dtypes.bfloat16
F32 = np.float32

DIM = 4096
SEQ = 2048
HD = 128
N_HEADS = 32
N_KV = 8
QH = 4          # q heads per core
NM = QH + 2     # 6 m-tiles per core: Q0..Q3 V K
TC = 512        # token chunk
NKK = DIM // 128      # 32 contraction chunks
NCH = 8               # (batch, qc) chunks per core
NPC = 4               # x pieces per chunk (8 kk each)
SCALE = 1.0 / float(np.sqrt(HD))

# de-interleave permutation within one head_dim: evens then odds
PERM = np.concatenate([np.arange(0, HD, 2), np.arange(1, HD, 2)])

_CACHE = {}
LAST_RESULT = None


def _emit(tc_ctx, nc, tens, phases=3, first=True):
    dt = mybir.dt
    AF = mybir.ActivationFunctionType
    xs, wd, wod, cosd, sind, trid, bqd, bkd, out = tens

    ctx = tc_ctx._emit_ctx
    cns = tc_ctx._emit_cns

    (w_res, wo_res, cos_sb, sin_sb, tri_sb, ident, ones_col, bq_sb,
     bk_sb) = cns["consts"]
    kvp = cns["kvp"]
    xring = cns["xring"]
    qring = cns["qring"]
    aring = cns["aring"]
    stage = cns["stage"]
    epool = cns["epool"]
    espool = cns["espool"]
    npool = cns["npool"]
    obpool = cns["obpool"]
    ps_ph1 = cns["ps_ph1"]
    ps_st = cns["ps_st"]
    ps_wk = cns["ps_wk"]

    k_sb = cns["k_sb"]    # [128, 2048] per batch
    v_sb = cns["v_sb"]    # [128, 16*128] per batch (block tt at cols tt*128)

    q_t = [None, None]    # chunk ring handles
    a_t = [None, None]

    def dma_x(ch):
        for p in range(NPC):
            xp = xring.tile([128, 8 * TC], dt.bfloat16, name="xp", tag="xp")
            nc.sync.dma_start(
                xp[:], xs[ch * 128:(ch + 1) * 128, p * 8 * TC:(p + 1) * 8 * TC])
            cns.setdefault("xp", {})[(ch % 2, p)] = xp

    def rope(pst, dst, bias_ap, qc):
        sl = slice(qc * TC, (qc + 1) * TC)
        pre = stage.tile([128, TC], dt.bfloat16, name="pre", tag="pre")
        nc.vector.tensor_scalar_add(pre[:], pst[:], bias_ap)
        rot = stage.tile([128, TC], dt.bfloat16, name="rot", tag="rot")
        nc.gpsimd.dma_start(rot[0:64, :], pre[64:128, :])
        nc.gpsimd.dma_start(rot[64:128, :], pre[0:64, :])
        nc.vector.tensor_mul(pre[:], pre[:], cos_sb[:, sl])
        nc.vector.tensor_mul(rot[:], rot[:], sin_sb[:, sl])
        nc.vector.tensor_add(dst[:], pre[:], rot[:])

    def ph1_sg(ch, sg):
        """Projection subgroup sg of chunk ch: m-tiles (2sg, 2sg+1)."""
        b, qc = ch // 4, ch % 4
        if sg == 0:
            q_t[ch % 2] = qring.tile([128, QH * TC], dt.bfloat16, name="qt",
                                     tag="qt")
        pss = [ps_ph1.tile([128, TC], dt.float32, name="acc", tag="acc")
               for _ in range(2)]
        for kk in range(NKK):
            xp = cns["xp"][(ch % 2, kk // 8)]
            rhs = xp[:, (kk % 8) * TC:(kk % 8 + 1) * TC]
            for j in range(2):
                m = 2 * sg + j
                nc.tensor.matmul(
                    pss[j][:], wd_sb(m, kk), rhs,
                    start=(kk == 0), stop=(kk == NKK - 1),
                    skip_group_check=True)
        for j in range(2):
            m = 2 * sg + j
            if m < QH:      # Q head
                rope(pss[j], q_t[ch % 2][:, m * TC:(m + 1) * TC],
                     bq_sb[:, m:m + 1], qc)
            elif m == QH:   # V
                vtmp = stage.tile([128, TC], dt.bfloat16, name="vtmp",
                                  tag="vtmp")
                nc.vector.tensor_copy(vtmp[:], pss[j][:])
                for tl in range(4):
                    pvt = ps_wk.tile([128, 128], dt.bfloat16, name="pvt",
                                     tag="wk")
                    nc.tensor.transpose(
                        pvt[:], vtmp[:, tl * 128:(tl + 1) * 128], ident[:])
                    tt = qc * 4 + tl
                    nc.scalar.copy(
                        v_sb[b][:, tt * 128:(tt + 1) * 128], pvt[:])
            else:           # K
                rope(pss[j], k_sb[b][:, qc * TC:(qc + 1) * TC],
                     bk_sb[:, 0:1], qc)

    def wd_sb(m, kk):
        return w_res[:, kk * NM * 128 + m * 128:kk * NM * 128 + (m + 1) * 128]

    def attn_head(ch, h, depth=3):
        b, qc = ch // 4, ch % 4
        nkt = 4 * qc + 4
        qt = q_t[ch % 2]

        def st_of(kt):
            o = (kt - 4 * qc) * 128 if kt >= 4 * qc else 0
            st = ps_st.tile([128, TC], dt.float32, name="st", tag="st")
            nc.tensor.matmul(
                st[:, o:TC], k_sb[b][:, kt * 128:(kt + 1) * 128],
                qt[:, h * TC + o:(h + 1) * TC], start=True, stop=True,
                skip_group_check=True)
            return st, o

        pv = ps_wk.tile([128, TC], dt.float32, name="pv", tag="wk")
        esum = espool.tile([128, TC], dt.bfloat16, name="esum", tag="esum")
        sts = [st_of(k) for k in range(min(depth, nkt))]
        for kt in range(nkt):
            if kt + depth < nkt:
                sts.append(st_of(kt + depth))
            st, o = sts[kt]
            e = epool.tile([128, TC], dt.bfloat16, name="expS", tag="e")
            nc.scalar.activation(e[:, o:TC], st[:, o:TC], AF.Exp, scale=SCALE)
            if kt >= 4 * qc:
                nc.vector.tensor_mul(e[:, o:o + 128], e[:, o:o + 128],
                                     tri_sb[:])
            if kt == 0:
                nc.gpsimd.tensor_copy(esum[:], e[:])
            else:
                nc.gpsimd.tensor_add(esum[:, o:TC], esum[:, o:TC], e[:, o:TC])
            nc.tensor.matmul(pv[:, o:TC],
                             v_sb[b][:, kt * 128:(kt + 1) * 128],
                             e[:, o:TC], start=(kt == 0),
                             stop=(kt == nkt - 1), skip_group_check=True)
        sums = ps_wk.tile([1, TC], dt.float32, name="sums", tag="wk")
        nc.tensor.matmul(sums[:], ones_col[:], esum[:], start=True, stop=True,
                         skip_group_check=True)
        recip = npool.tile([1, TC], dt.float32, name="recip", tag="recip")
        nc.vector.reciprocal(recip[:], sums[:])
        bcs = npool.tile([128, TC], dt.float32, name="bcs", tag="bcs")
        nc.gpsimd.partition_broadcast(bcs[:], recip[:])
        nc.vector.tensor_mul(a_t[ch % 2][:, h * TC:(h + 1) * TC], pv[:],
                             bcs[:])

    def ph3(ch):
        b, qc = ch // 4, ch % 4
        at = a_t[ch % 2]
        n = 0
        for tl in range(4):
            for oc in range(8):
                po = ps_wk.tile([128, TC], dt.float32, name="po", tag="wk")
                for h in range(QH):
                    nc.tensor.matmul(
                        po[:], at[:, h * TC + tl * 128:h * TC + tl * 128 + 128],
                        wo_res[:, h * DIM + oc * TC:h * DIM + (oc + 1) * TC],
                        start=(h == 0), stop=(h == QH - 1),
                        skip_group_check=True)
                ob = obpool.tile([128, TC], dt.float32, name="ob", tag="ob")
                if n % 2 == 0:
                    nc.vector.tensor_copy(ob[:], po[:])
                else:
                    nc.scalar.copy(ob[:], po[:])
                eng = nc.sync if n % 2 == 0 else nc.scalar
                eng.dma_start(
                    out[b * SEQ + qc * TC + tl * 128:
                        b * SEQ + qc * TC + tl * 128 + 128,
                        oc * TC:(oc + 1) * TC], ob[:])
                n += 1

    # ---------------- body ----------------
    if phases == 1:
        for ch in range(NCH):
            dma_x(ch)
            a_t[ch % 2] = aring.tile([128, QH * TC], dt.bfloat16, name="at",
                                     tag="at")
            for sg in range(3):
                ph1_sg(ch, sg)
        for b in range(2):
            nc.sync.dma_start(out[b * SEQ:b * SEQ + 128, 0:1024],
                              k_sb[b][:, :].bitcast(dt.float32))
            nc.sync.dma_start(out[b * SEQ + 128:b * SEQ + 256, 0:1024],
                              v_sb[b][:, :].bitcast(dt.float32))
        for r in range(2):
            if q_t[r] is not None:
                nc.sync.dma_start(out[r * 128:(r + 1) * 128, 1024:2048],
                                  q_t[r][:, :].bitcast(dt.float32))
        return

    dma_x(0)
    a_t[0] = aring.tile([128, QH * TC], dt.bfloat16, name="at", tag="at")
    for sg in range(3):
        ph1_sg(0, sg)
    for i in range(NCH):
        if i < NCH - 1:
            dma_x(i + 1)
            a_t[(i + 1) % 2] = aring.tile([128, QH * TC], dt.bfloat16,
                                          name="at", tag="at")
        for h in range(QH):
            attn_head(i, h)
            if i < NCH - 1:
                if h < 2:
                    ph1_sg(i + 1, h)
                elif h == 3:
                    ph1_sg(i + 1, 2)
        if phases >= 3:
            ph3(i)
    if phases == 2:
        for r in range(2):
            nc.sync.dma_start(out[r * 128:(r + 1) * 128, 2048:3072],
                              a_t[r][:, :].bitcast(dt.float32))


def build_nc(num_devices=8, reps=1, phases=3):
    nc = bacc.Bacc("TRN2", target_bir_lowering=False, debug=False,
                   enable_asserts=False, num_devices=num_devices)
    dt = mybir.dt
    xs = nc.dram_tensor("xs", [NCH * 128, NKK * TC], dt.bfloat16,
                        kind="ExternalInput").ap()
    wd = nc.dram_tensor("wT", [128, NKK * NM * 128], dt.bfloat16,
                        kind="ExternalInput").ap()
    wod = nc.dram_tensor("woT", [128, QH * DIM], dt.bfloat16,
                         kind="ExternalInput").ap()
    cosd = nc.dram_tensor("cos128", [HD, SEQ], dt.bfloat16,
                          kind="ExternalInput").ap()
    sind = nc.dram_tensor("sin128s", [HD, SEQ], dt.bfloat16,
                          kind="ExternalInput").ap()
    trid = nc.dram_tensor("tri", [128, 128], dt.bfloat16,
                          kind="ExternalInput").ap()
    bqd = nc.dram_tensor("bq_sb", [HD, QH], dt.float32,
                         kind="ExternalInput").ap()
    bkd = nc.dram_tensor("bk_sb", [HD, 1], dt.float32,
                         kind="ExternalInput").ap()
    out = nc.dram_tensor("out", [2 * SEQ, DIM], dt.float32,
                         kind="ExternalOutput").ap()
    tens = (xs, wd, wod, cosd, sind, trid, bqd, bkd, out)

    with tile.TileContext(nc) as tctx:
        with ExitStack() as ctx:
            cns = {}
            tctx._emit_ctx = ctx
            tctx._emit_cns = cns
            cpool = ctx.enter_context(tc_pool(tctx, "consts", 1))
            w_res = cpool.tile([128, NKK * NM * 128], dt.bfloat16, name="w_res")
            wo_res = cpool.tile([128, QH * DIM], dt.bfloat16, name="wo_res")
            cos_sb = cpool.tile([HD, SEQ], dt.bfloat16, name="cos_sb")
            sin_sb = cpool.tile([HD, SEQ], dt.bfloat16, name="sin_sb")
            tri_sb = cpool.tile([128, 128], dt.bfloat16, name="tri_sb")
            ident = cpool.tile([128, 128], dt.bfloat16, name="ident")
            ones_col = cpool.tile([128, 1], dt.bfloat16, name="ones_col")
            bq_sb = cpool.tile([HD, QH], dt.float32, name="bq_sb")
            bk_sb = cpool.tile([HD, 1], dt.float32, name="bk_sb")
            cns["consts"] = (w_res, wo_res, cos_sb, sin_sb, tri_sb, ident,
                             ones_col, bq_sb, bk_sb)
            kvpool = ctx.enter_context(tc_pool(tctx, "kvp", 1))
            cns["kvp"] = kvpool
            cns["k_sb"] = [kvpool.tile([128, SEQ], dt.bfloat16, name=f"k{b}")
                           for b in range(2)]
            cns["v_sb"] = [kvpool.tile([128, SEQ], dt.bfloat16, name=f"v{b}")
                           for b in range(2)]
            cns["xring"] = ctx.enter_context(tc_pool(tctx, "xr", 6))
            cns["qring"] = ctx.enter_context(tc_pool(tctx, "qr", 2))
            cns["aring"] = ctx.enter_context(tc_pool(tctx, "ar", 2))
            cns["stage"] = ctx.enter_context(tc_pool(tctx, "stg", 2))
            cns["epool"] = ctx.enter_context(tc_pool(tctx, "ep", 8))
            cns["espool"] = ctx.enter_context(tc_pool(tctx, "esp", 2))
            cns["npool"] = ctx.enter_context(tc_pool(tctx, "np", 2))
            cns["obpool"] = ctx.enter_context(tc_pool(tctx, "obp", 4))
            cns["ps_ph1"] = ctx.enter_context(
                tc_pool(tctx, "psph1", 2, space="PSUM"))
            cns["ps_st"] = ctx.enter_context(
                tc_pool(tctx, "psst", 3, space="PSUM"))
            cns["ps_wk"] = ctx.enter_context(
                tc_pool(tctx, "pswk", 3, space="PSUM"))

            # one-time constant loads (outside the reps loop)
            nc.scalar.dma_start(w_res[:, 0:NKK * NM * 32],
                                wd[:, 0:NKK * NM * 32])
            nc.scalar.dma_start(w_res[:, NKK * NM * 32:NKK * NM * 64],
                                wd[:, NKK * NM * 32:NKK * NM * 64])
            nc.scalar.dma_start(w_res[:, NKK * NM * 64:NKK * NM * 96],
                                wd[:, NKK * NM * 64:NKK * NM * 96])
            nc.scalar.dma_start(w_res[:, NKK * NM * 96:NKK * NM * 128],
                                wd[:, NKK * NM * 96:NKK * NM * 128])
            nc.scalar.dma_start(wo_res[:], wod[:])
            nc.scalar.dma_start(cos_sb[:], cosd[:])
            nc.scalar.dma_start(sin_sb[:], sind[:])
            nc.scalar.dma_start(tri_sb[:], trid[:])
            nc.scalar.dma_start(bq_sb[:], bqd[:])
            nc.scalar.dma_start(bk_sb[:], bkd[:])
            nc.vector.memset(ones_col[:], 1.0)
            masks_mod.make_identity(nc, ident[:])

            if reps == 1:
                _emit(tctx, nc, tens, phases=phases)
            else:
                with tctx.For_i(0, reps, 1):
                    _emit(tctx, nc, tens, phases=phases)
    nc.compile()
    return nc


def tc_pool(tctx, name, bufs, space="SBUF"):
    return tctx.tile_pool(name=name, bufs=bufs, space=space)


def _get_nc():
    if "nc" not in _CACHE:
        _CACHE["nc"] = build_nc()
    return _CACHE["nc"]


def make_in_maps(x, freqs_cos, freqs_sin, mask, wq, bq, wk, bk, wv, bv,
                 wo, bo=None):
    x = np.asarray(x, F32)
    # xs: [8 chunks, 128, 32 kk * 512] with 32KB contiguous rows
    X = np.zeros((NCH, 128, NKK * TC), F32)
    for b in range(2):
        xT = np.ascontiguousarray(x[b].T)  # [4096, 2048]
        blk = xT.reshape(NKK, 128, 4, TC).transpose(2, 1, 0, 3)  # [qc,128,kk,TC]
        X[b * 4:(b + 1) * 4] = blk.reshape(4, 128, NKK * TC)
    xs = np.ascontiguousarray(X.reshape(NCH * 128, NKK * TC)).astype(BF16)

    cosT = np.asarray(freqs_cos, F32).T  # [64, 2048]
    sinT = np.asarray(freqs_sin, F32).T
    cos128 = np.ascontiguousarray(np.vstack([cosT, cosT])).astype(BF16)
    sin128s = np.ascontiguousarray(np.vstack([-sinT, sinT])).astype(BF16)
    mask = np.asarray(mask, F32)
    tri = np.ascontiguousarray((mask[0:128, 0:128] == 0).T).astype(BF16)

    wq, wk, wv, wo = (np.asarray(a, F32) for a in (wq, wk, wv, wo))
    bq, bk = np.asarray(bq, F32), np.asarray(bk, F32)
    in_maps = []
    for c in range(8):
        qrows = wq[c * 512:(c + 1) * 512].reshape(QH, HD, DIM)[:, PERM, :]
        vrows = wv[c * 128:(c + 1) * 128]
        krows = wk[c * 128:(c + 1) * 128][PERM, :]
        wcat = np.concatenate([qrows.reshape(QH * HD, DIM), vrows, krows],
                              axis=0)  # [768, 4096] m-order Q0..Q3 V K
        wT_c = wcat.T  # [4096, 768]
        W = np.ascontiguousarray(
            wT_c.reshape(NKK, 128, NM * 128).transpose(1, 0, 2)
            .reshape(128, NKK * NM * 128)).astype(BF16)
        woT_c = wo[:, c * 512:(c + 1) * 512].T  # [512, 4096]
        WO = np.ascontiguousarray(
            woT_c.reshape(QH, 128, DIM).transpose(1, 0, 2)
            .reshape(128, QH * DIM)).astype(BF16)
        bq_c = np.ascontiguousarray(
            bq[c * 512:(c + 1) * 512].reshape(QH, HD)[:, PERM].T).astype(F32)
        bk_c = np.ascontiguousarray(
            bk[c * 128:(c + 1) * 128][PERM].reshape(HD, 1)).astype(F32)
        in_maps.append(dict(xs=xs, wT=W, woT=WO, cos128=cos128,
                            sin128s=sin128s, tri=tri, bq_sb=bq_c, bk_sb=bk_c))
    return in_maps


def kernel(x, freqs_cos, freqs_sin, mask, wq, bq, wk, bk, wv, bv, wo, bo):
    global LAST_RESULT
    nc = _get_nc()
    in_maps = make_in_maps(x, freqs_cos, freqs_sin, mask, wq, bq, wk, bk,
                           wv, bv, wo, bo)
    res = run_bass_kernel_spmd(nc, in_maps, core_ids=list(range(8)))
    LAST_RESULT = res
    outp = np.zeros((2, SEQ, DIM), F32)
    for c in range(8):
        part = np.asarray(res.results[c]["out"], F32)
        outp[0] += part[0:SEQ]
        outp[1] += part[SEQ:2 * SEQ]
    bv = np.asarray(bv, F32)
    bo = np.asarray(bo, F32)
    wo = np.asarray(wo, F32)
    bv_exp = np.broadcast_to(
        bv.reshape(N_KV, 1, HD), (N_KV, N_HEADS // N_KV, HD)).reshape(DIM)
    outp += (bv_exp @ wo.T + bo)[None, None, :].astype(F32)
    return outp


# revision 9
# speedup vs baseline: 1.3173x; 1.3173x over previous
"""Trainium2 Bass kernel for GQA attention prefill (Llama-style).

Reference computation (fp32):
  xq = x@wq.T+bq; xk = x@wk.T+bk; xv = x@wv.T+bv
  rope(xq, xk); scores = q@k.T/sqrt(128) + causal_mask
  probs = softmax(scores); out = (probs@v) reshaped @ wo.T + bo

Shapes: x [2, 2048, 4096], 32 q heads / 8 kv heads, head_dim 128.

Sharding: TP=8 over head groups — core c owns q heads 4c..4c+3 and kv head c
(GQA group = exactly one core), both batches. Each core computes a partial
[2*2048, 4096] output (its heads' contribution through its wo columns); the
host sums the 8 partials per batch. bq/bk applied on device; bv/bo folded
into a constant host-side row (sum_k probs == 1).

Single fused pass per core, software-pipelined over 8 token chunks
(batch, 512-token qc). Per chunk: QKV projection (+rope) -> attention for
the 4 heads -> output projection of those 512 rows. The next chunk's
projection matmuls are interleaved between attention heads so the PE never
waits on the exp (ACT) engine, and weights (w, wo) stay SBUF-resident the
whole kernel. x is streamed via a host-side relayout that gives 8 KB
contiguous lines per partition (full DMA efficiency).

Layouts on device (all matmuls bf16, fp32 PSUM):
  - projections produce q/k/v in [head_dim, tok]; rope via de-interleaved
    head_dim (host permutes wq/wk rows: evens then odds), rotation = swap of
    64-partition halves via SBUF-SBUF DMA, sign folded into sin.
  - scores transposed S_T[k, q]; softmax denominator = DVE-accumulated
    exp-sum reduced by one ones-column matmul per (head, chunk).
  - causal masking: column-trimmed score blocks + one [128,128] triangular
    binary multiply per diagonal block.
  - PV output lands in [head_dim, tok] == lhsT of the wo matmul.
"""
import sys

for _p in ("/opt/trn_rl_repo",):
    if _p not in sys.path:
        sys.path.insert(0, _p)

from contextlib import ExitStack

import ml_dtypes
import numpy as np

import concourse.bass as bass  # noqa: F401  (AP types used implicitly)
import concourse.tile as tile
from concourse import bacc, mybir
from concourse import masks as masks_mod
from concourse.bass_utils import run_bass_kernel_spmd

BF16 = ml_dtypes.bfloat16
F32 = np.float32

DIM = 4096
SEQ = 2048
HD = 128
N_HEADS = 32
N_KV = 8
QH = 4          # q heads per core
NM = QH + 2     # 6 m-tiles per core: Q0..Q3 V K
TC = 512        # token chunk
NKK = DIM // 128      # 32 contraction chunks
NCH = 8               # (batch, qc) chunks per core
NPC = 4               # x pieces per chunk (8 kk each)
SCALE = 1.0 / float(np.sqrt(HD))

# de-interleave permutation within one head_dim: evens then odds
PERM = np.concatenate([np.arange(0, HD, 2), np.arange(1, HD, 2)])

_CACHE = {}
LAST_RESULT = None


def _emit(tc_ctx, nc, tens, phases=3, first=True):
    dt = mybir.dt
    AF = mybir.ActivationFunctionType
    xs, wd, wod, cosd, sind, trid, bqd, bkd, out = tens

    ctx = tc_ctx._emit_ctx
    cns = tc_ctx._emit_cns

    (w_res, wo_res, cos_sb, sin_sb, tri_sb, ident, ones_col, bq_sb,
     bk_sb) = cns["consts"]
    kvp = cns["kvp"]
    xring = cns["xring"]
    qring = cns["qring"]
    aring = cns["aring"]
    stage = cns["stage"]
    epool = cns["epool"]
    espool = cns["espool"]
    npool = cns["npool"]
    obpool = cns["obpool"]
    ps_ph1 = cns["ps_ph1"]
    ps_st = cns["ps_st"]
    ps_wk = cns["ps_wk"]

    k_sb = cns["k_sb"]    # [128, 2048] per batch
    v_sb = cns["v_sb"]    # [128, 16*128] per batch (block tt at cols tt*128)

    q_t = [None, None]    # chunk ring handles
    a_t = [None, None]

    def dma_x(ch):
        for p in range(NPC):
            xp = xring.tile([128, 8 * TC], dt.bfloat16, name="xp", tag="xp")
            nc.sync.dma_start(
                xp[:], xs[ch * 128:(ch + 1) * 128, p * 8 * TC:(p + 1) * 8 * TC])
            cns.setdefault("xp", {})[(ch % 2, p)] = xp

    def rope(pst, dst, bias_ap, qc):
        sl = slice(qc * TC, (qc + 1) * TC)
        pre = stage.tile([128, TC], dt.bfloat16, name="pre", tag="pre")
        nc.vector.tensor_scalar_add(pre[:], pst[:], bias_ap)
        rot = stage.tile([128, TC], dt.bfloat16, name="rot", tag="rot")
        nc.gpsimd.dma_start(rot[0:64, :], pre[64:128, :])
        nc.gpsimd.dma_start(rot[64:128, :], pre[0:64, :])
        nc.vector.tensor_mul(pre[:], pre[:], cos_sb[:, sl])
        nc.vector.tensor_mul(rot[:], rot[:], sin_sb[:, sl])
        nc.vector.tensor_add(dst[:], pre[:], rot[:])

    SGS = ((0, 1), (2, 3), (4,), (5,))  # m-tiles per subgroup: Q01 Q23 V K

    def ph1_sg(ch, sg):
        """Projection subgroup sg of chunk ch."""
        b, qc = ch // 4, ch % 4
        if sg == 0:
            q_t[ch % 2] = qring.tile([128, QH * TC], dt.bfloat16, name="qt",
                                     tag="qt")
        ms = SGS[sg]
        pss = [ps_ph1.tile([128, TC], dt.float32, name="acc", tag="acc")
               for _ in ms]
        for kk in range(NKK):
            xp = cns["xp"][(ch % 2, kk // 8)]
            rhs = xp[:, (kk % 8) * TC:(kk % 8 + 1) * TC]
            for j, m in enumerate(ms):
                nc.tensor.matmul(
                    pss[j][:], wd_sb(m, kk), rhs,
                    start=(kk == 0), stop=(kk == NKK - 1),
                    skip_group_check=True)
        for j, m in enumerate(ms):
            if m < QH:      # Q head
                rope(pss[j], q_t[ch % 2][:, m * TC:(m + 1) * TC],
                     bq_sb[:, m:m + 1], qc)
            elif m == QH:   # V
                vtmp = stage.tile([128, TC], dt.bfloat16, name="vtmp",
                                  tag="vtmp")
                nc.vector.tensor_copy(vtmp[:], pss[j][:])
                for tl in range(4):
                    pvt = ps_wk.tile([128, 128], dt.bfloat16, name="pvt",
                                     tag="wk")
                    nc.tensor.transpose(
                        pvt[:], vtmp[:, tl * 128:(tl + 1) * 128], ident[:])
                    tt = qc * 4 + tl
                    nc.scalar.copy(
                        v_sb[b][:, tt * 128:(tt + 1) * 128], pvt[:])
            else:           # K
                rope(pss[j], k_sb[b][:, qc * TC:(qc + 1) * TC],
                     bk_sb[:, 0:1], qc)

    def wd_sb(m, kk):
        return w_res[:, kk * NM * 128 + m * 128:kk * NM * 128 + (m + 1) * 128]

    def attn_head(ch, h, depth=3):
        b, qc = ch // 4, ch % 4
        nkt = 4 * qc + 4
        qt = q_t[ch % 2]

        def st_of(kt):
            o = (kt - 4 * qc) * 128 if kt >= 4 * qc else 0
            st = ps_st.tile([128, TC], dt.float32, name="st", tag="st")
            nc.tensor.matmul(
                st[:, o:TC], k_sb[b][:, kt * 128:(kt + 1) * 128],
                qt[:, h * TC + o:(h + 1) * TC], start=True, stop=True,
                skip_group_check=True)
            return st, o

        pv = ps_wk.tile([128, TC], dt.float32, name="pv", tag="wk")
        esum = espool.tile([128, TC], dt.bfloat16, name="esum", tag="esum")
        sts = [st_of(k) for k in range(min(depth, nkt))]
        for kt in range(nkt):
            if kt + depth < nkt:
                sts.append(st_of(kt + depth))
            st, o = sts[kt]
            e = epool.tile([128, TC], dt.bfloat16, name="expS", tag="e")
            nc.scalar.activation(e[:, o:TC], st[:, o:TC], AF.Exp, scale=SCALE)
            if kt >= 4 * qc:
                nc.vector.tensor_mul(e[:, o:o + 128], e[:, o:o + 128],
                                     tri_sb[:])
            if kt == 0:
                nc.vector.tensor_copy(esum[:], e[:])
            elif kt >= 4 * qc:
                # diag blocks: esum add off the DVE queue (DVE does tri-mul,
                # which gates the pv matmul; esum never gates pv)
                nc.gpsimd.tensor_add(esum[:, o:TC], esum[:, o:TC], e[:, o:TC])
            else:
                nc.vector.tensor_add(esum[:, o:TC], esum[:, o:TC], e[:, o:TC])
            nc.tensor.matmul(pv[:, o:TC],
                             v_sb[b][:, kt * 128:(kt + 1) * 128],
                             e[:, o:TC], start=(kt == 0),
                             stop=(kt == nkt - 1), skip_group_check=True)
        sums = ps_wk.tile([1, TC], dt.float32, name="sums", tag="wk")
        nc.tensor.matmul(sums[:], ones_col[:], esum[:], start=True, stop=True,
                         skip_group_check=True)
        recip = npool.tile([1, TC], dt.float32, name="recip", tag="recip")
        nc.vector.reciprocal(recip[:], sums[:])
        bcs = npool.tile([128, TC], dt.float32, name="bcs", tag="bcs")
        nc.gpsimd.partition_broadcast(bcs[:], recip[:])
        nc.vector.tensor_mul(a_t[ch % 2][:, h * TC:(h + 1) * TC], pv[:],
                             bcs[:])

    def ph3(ch):
        b, qc = ch // 4, ch % 4
        at = a_t[ch % 2]
        n = 0
        for tl in range(4):
            for oc in range(8):
                po = ps_wk.tile([128, TC], dt.float32, name="po", tag="wk")
                for h in range(QH):
                    nc.tensor.matmul(
                        po[:], at[:, h * TC + tl * 128:h * TC + tl * 128 + 128],
                        wo_res[:, h * DIM + oc * TC:h * DIM + (oc + 1) * TC],
                        start=(h == 0), stop=(h == QH - 1),
                        skip_group_check=True)
                ob = obpool.tile([128, TC], dt.float32, name="ob", tag="ob")
                if n % 2 == 0:
                    nc.vector.tensor_copy(ob[:], po[:])
                else:
                    nc.scalar.copy(ob[:], po[:])
                eng = nc.sync if n % 2 == 0 else nc.scalar
                eng.dma_start(
                    out[b * SEQ + qc * TC + tl * 128:
                        b * SEQ + qc * TC + tl * 128 + 128,
                        oc * TC:(oc + 1) * TC], ob[:])
                n += 1

    # ---------------- body ----------------
    if phases == 1:
        for ch in range(NCH):
            dma_x(ch)
            a_t[ch % 2] = aring.tile([128, QH * TC], dt.bfloat16, name="at",
                                     tag="at")
            for sg in range(4):
                ph1_sg(ch, sg)
        for b in range(2):
            nc.sync.dma_start(out[b * SEQ:b * SEQ + 128, 0:1024],
                              k_sb[b][:, :].bitcast(dt.float32))
            nc.sync.dma_start(out[b * SEQ + 128:b * SEQ + 256, 0:1024],
                              v_sb[b][:, :].bitcast(dt.float32))
        for r in range(2):
            if q_t[r] is not None:
                nc.sync.dma_start(out[r * 128:(r + 1) * 128, 1024:2048],
                                  q_t[r][:, :].bitcast(dt.float32))
        return

    dma_x(0)
    a_t[0] = aring.tile([128, QH * TC], dt.bfloat16, name="at", tag="at")
    for sg in range(4):
        ph1_sg(0, sg)
    for i in range(NCH):
        if i < NCH - 1:
            dma_x(i + 1)
            a_t[(i + 1) % 2] = aring.tile([128, QH * TC], dt.bfloat16,
                                          name="at", tag="at")
        for h in range(QH):
            attn_head(i, h)
            if i < NCH - 1:
                ph1_sg(i + 1, h)
        if phases >= 3:
            ph3(i)
    if phases == 2:
        for r in range(2):
            nc.sync.dma_start(out[r * 128:(r + 1) * 128, 2048:3072],
                              a_t[r][:, :].bitcast(dt.float32))


def build_nc(num_devices=8, reps=1, phases=3):
    nc = bacc.Bacc("TRN2", target_bir_lowering=False, debug=False,
                   enable_asserts=False, num_devices=num_devices)
    dt = mybir.dt
    xs = nc.dram_tensor("xs", [NCH * 128, NKK * TC], dt.bfloat16,
                        kind="ExternalInput").ap()
    wd = nc.dram_tensor("wT", [128, NKK * NM * 128], dt.bfloat16,
                        kind="ExternalInput").ap()
    wod = nc.dram_tensor("woT", [128, QH * DIM], dt.bfloat16,
                         kind="ExternalInput").ap()
    cosd = nc.dram_tensor("cos128", [HD, SEQ], dt.bfloat16,
                          kind="ExternalInput").ap()
    sind = nc.dram_tensor("sin128s", [HD, SEQ], dt.bfloat16,
                          kind="ExternalInput").ap()
    trid = nc.dram_tensor("tri", [128, 128], dt.bfloat16,
                          kind="ExternalInput").ap()
    bqd = nc.dram_tensor("bq_sb", [HD, QH], dt.float32,
                         kind="ExternalInput").ap()
    bkd = nc.dram_tensor("bk_sb", [HD, 1], dt.float32,
                         kind="ExternalInput").ap()
    out = nc.dram_tensor("out", [2 * SEQ, DIM], dt.float32,
                         kind="ExternalOutput").ap()
    tens = (xs, wd, wod, cosd, sind, trid, bqd, bkd, out)

    with tile.TileContext(nc) as tctx:
        with ExitStack() as ctx:
            cns = {}
            tctx._emit_ctx = ctx
            tctx._emit_cns = cns
            cpool = ctx.enter_context(tc_pool(tctx, "consts", 1))
            w_res = cpool.tile([128, NKK * NM * 128], dt.bfloat16, name="w_res")
            wo_res = cpool.tile([128, QH * DIM], dt.bfloat16, name="wo_res")
            cos_sb = cpool.tile([HD, SEQ], dt.bfloat16, name="cos_sb")
            sin_sb = cpool.tile([HD, SEQ], dt.bfloat16, name="sin_sb")
            tri_sb = cpool.tile([128, 128], dt.bfloat16, name="tri_sb")
            ident = cpool.tile([128, 128], dt.bfloat16, name="ident")
            ones_col = cpool.tile([128, 1], dt.bfloat16, name="ones_col")
            bq_sb = cpool.tile([HD, QH], dt.float32, name="bq_sb")
            bk_sb = cpool.tile([HD, 1], dt.float32, name="bk_sb")
            cns["consts"] = (w_res, wo_res, cos_sb, sin_sb, tri_sb, ident,
                             ones_col, bq_sb, bk_sb)
            kvpool = ctx.enter_context(tc_pool(tctx, "kvp", 1))
            cns["kvp"] = kvpool
            cns["k_sb"] = [kvpool.tile([128, SEQ], dt.bfloat16, name=f"k{b}")
                           for b in range(2)]
            cns["v_sb"] = [kvpool.tile([128, SEQ], dt.bfloat16, name=f"v{b}")
                           for b in range(2)]
            cns["xring"] = ctx.enter_context(tc_pool(tctx, "xr", 6))
            cns["qring"] = ctx.enter_context(tc_pool(tctx, "qr", 2))
            cns["aring"] = ctx.enter_context(tc_pool(tctx, "ar", 2))
            cns["stage"] = ctx.enter_context(tc_pool(tctx, "stg", 2))
            cns["epool"] = ctx.enter_context(tc_pool(tctx, "ep", 8))
            cns["espool"] = ctx.enter_context(tc_pool(tctx, "esp", 2))
            cns["npool"] = ctx.enter_context(tc_pool(tctx, "np", 2))
            cns["obpool"] = ctx.enter_context(tc_pool(tctx, "obp", 4))
            cns["ps_ph1"] = ctx.enter_context(
                tc_pool(tctx, "psph1", 2, space="PSUM"))
            cns["ps_st"] = ctx.enter_context(
                tc_pool(tctx, "psst", 3, space="PSUM"))
            cns["ps_wk"] = ctx.enter_context(
                tc_pool(tctx, "pswk", 3, space="PSUM"))

            # one-time constant loads (outside the reps loop)
            nc.scalar.dma_start(w_res[:, 0:NKK * NM * 32],
                                wd[:, 0:NKK * NM * 32])
            nc.scalar.dma_start(w_res[:, NKK * NM * 32:NKK * NM * 64],
                                wd[:, NKK * NM * 32:NKK * NM * 64])
            nc.scalar.dma_start(w_res[:, NKK * NM * 64:NKK * NM * 96],
                                wd[:, NKK * NM * 64:NKK * NM * 96])
            nc.scalar.dma_start(w_res[:, NKK * NM * 96:NKK * NM * 128],
                                wd[:, NKK * NM * 96:NKK * NM * 128])
            nc.scalar.dma_start(wo_res[:], wod[:])
            nc.scalar.dma_start(cos_sb[:], cosd[:])
            nc.scalar.dma_start(sin_sb[:], sind[:])
            nc.scalar.dma_start(tri_sb[:], trid[:])
            nc.scalar.dma_start(bq_sb[:], bqd[:])
            nc.scalar.dma_start(bk_sb[:], bkd[:])
            nc.vector.memset(ones_col[:], 1.0)
            masks_mod.make_identity(nc, ident[:])

            if reps == 1:
                _emit(tctx, nc, tens, phases=phases)
            else:
                with tctx.For_i(0, reps, 1):
                    _emit(tctx, nc, tens, phases=phases)
    nc.compile()
    return nc


def tc_pool(tctx, name, bufs, space="SBUF"):
    return tctx.tile_pool(name=name, bufs=bufs, space=space)


def _get_nc():
    if "nc" not in _CACHE:
        _CACHE["nc"] = build_nc()
    return _CACHE["nc"]


def make_in_maps(x, freqs_cos, freqs_sin, mask, wq, bq, wk, bk, wv, bv,
                 wo, bo=None):
    x = np.asarray(x, F32)
    # xs: [8 chunks, 128, 32 kk * 512] with 32KB contiguous rows
    X = np.zeros((NCH, 128, NKK * TC), F32)
    for b in range(2):
        xT = np.ascontiguousarray(x[b].T)  # [4096, 2048]
        blk = xT.reshape(NKK, 128, 4, TC).transpose(2, 1, 0, 3)  # [qc,128,kk,TC]
        X[b * 4:(b + 1) * 4] = blk.reshape(4, 128, NKK * TC)
    xs = np.ascontiguousarray(X.reshape(NCH * 128, NKK * TC)).astype(BF16)

    cosT = np.asarray(freqs_cos, F32).T  # [64, 2048]
    sinT = np.asarray(freqs_sin, F32).T
    cos128 = np.ascontiguousarray(np.vstack([cosT, cosT])).astype(BF16)
    sin128s = np.ascontiguousarray(np.vstack([-sinT, sinT])).astype(BF16)
    mask = np.asarray(mask, F32)
    tri = np.ascontiguousarray((mask[0:128, 0:128] == 0).T).astype(BF16)

    wq, wk, wv, wo = (np.asarray(a, F32) for a in (wq, wk, wv, wo))
    bq, bk = np.asarray(bq, F32), np.asarray(bk, F32)
    in_maps = []
    for c in range(8):
        qrows = wq[c * 512:(c + 1) * 512].reshape(QH, HD, DIM)[:, PERM, :]
        vrows = wv[c * 128:(c + 1) * 128]
        krows = wk[c * 128:(c + 1) * 128][PERM, :]
        wcat = np.concatenate([qrows.reshape(QH * HD, DIM), vrows, krows],
                              axis=0)  # [768, 4096] m-order Q0..Q3 V K
        wT_c = wcat.T  # [4096, 768]
        W = np.ascontiguousarray(
            wT_c.reshape(NKK, 128, NM * 128).transpose(1, 0, 2)
            .reshape(128, NKK * NM * 128)).astype(BF16)
        woT_c = wo[:, c * 512:(c + 1) * 512].T  # [512, 4096]
        WO = np.ascontiguousarray(
            woT_c.reshape(QH, 128, DIM).transpose(1, 0, 2)
            .reshape(128, QH * DIM)).astype(BF16)
        bq_c = np.ascontiguousarray(
            bq[c * 512:(c + 1) * 512].reshape(QH, HD)[:, PERM].T).astype(F32)
        bk_c = np.ascontiguousarray(
            bk[c * 128:(c + 1) * 128][PERM].reshape(HD, 1)).astype(F32)
        in_maps.append(dict(xs=xs, wT=W, woT=WO, cos128=cos128,
                            sin128s=sin128s, tri=tri, bq_sb=bq_c, bk_sb=bk_c))
    return in_maps


def kernel(x, freqs_cos, freqs_sin, mask, wq, bq, wk, bk, wv, bv, wo, bo):
    global LAST_RESULT
    nc = _get_nc()
    in_maps = make_in_maps(x, freqs_cos, freqs_sin, mask, wq, bq, wk, bk,
                           wv, bv, wo, bo)
    res = run_bass_kernel_spmd(nc, in_maps, core_ids=list(range(8)))
    LAST_RESULT = res
    outp = np.zeros((2, SEQ, DIM), F32)
    for c in range(8):
        part = np.asarray(res.results[c]["out"], F32)
        outp[0] += part[0:SEQ]
        outp[1] += part[SEQ:2 * SEQ]
    bv = np.asarray(bv, F32)
    bo = np.asarray(bo, F32)
    wo = np.asarray(wo, F32)
    bv_exp = np.broadcast_to(
        bv.reshape(N_KV, 1, HD), (N_KV, N_HEADS // N_KV, HD)).reshape(DIM)
    outp += (bv_exp @ wo.T + bo)[None, None, :].astype(F32)
    return outp


# revision 10
# speedup vs baseline: 3.2869x; 2.4952x over previous
"""Trainium2 Bass kernel for GQA attention prefill (Llama-style).

Reference computation (fp32):
  xq = x@wq.T+bq; xk = x@wk.T+bk; xv = x@wv.T+bv
  rope(xq, xk); scores = q@k.T/sqrt(128) + causal_mask
  probs = softmax(scores); out = (probs@v) reshaped @ wo.T + bo

Shapes: x [2, 2048, 4096], 32 q heads / 8 kv heads, head_dim 128.

Sharding: TP=8 over head groups — core c owns q heads 4c..4c+3 and kv head c
(GQA group = exactly one core), both batches. Each core computes a partial
[2*2048, 4096] output (its heads' contribution through its wo columns); the
host sums the 8 partials per batch. bq/bk applied on device; bv/bo folded
into a constant host-side row (sum_k probs == 1).

Single fused pass per core, software-pipelined over 8 token chunks
(batch, 512-token qc). Per chunk: QKV projection (+rope) -> attention for
the 4 heads -> output projection of those 512 rows. The next chunk's
projection matmuls are interleaved between attention heads so the PE never
waits on the exp (ACT) engine, and weights (w, wo) stay SBUF-resident the
whole kernel. x is streamed via a host-side relayout that gives 8 KB
contiguous lines per partition (full DMA efficiency).

Layouts on device (all matmuls bf16, fp32 PSUM):
  - projections produce q/k/v in [head_dim, tok]; rope via de-interleaved
    head_dim (host permutes wq/wk rows: evens then odds), rotation = swap of
    64-partition halves via SBUF-SBUF DMA, sign folded into sin.
  - scores transposed S_T[k, q]; softmax denominator = DVE-accumulated
    exp-sum reduced by one ones-column matmul per (head, chunk).
  - causal masking: column-trimmed score blocks + one [128,128] triangular
    binary multiply per diagonal block.
  - PV output lands in [head_dim, tok] == lhsT of the wo matmul.
"""
import sys

for _p in ("/opt/trn_rl_repo",):
    if _p not in sys.path:
        sys.path.insert(0, _p)

from contextlib import ExitStack

import ml_dtypes
import numpy as np

import concourse.bass as bass  # noqa: F401  (AP types used implicitly)
import concourse.tile as tile
from concourse import bacc, mybir
from concourse import masks as masks_mod
from concourse.bass_utils import run_bass_kernel_spmd

BF16 = ml_dtypes.bfloat16
F32 = np.float32

DIM = 4096
SEQ = 2048
HD = 128
N_HEADS = 32
N_KV = 8
QH = 4          # q heads per core
NM = QH + 2     # 6 m-tiles per core: Q0..Q3 V K
TC = 512        # token chunk
NKK = DIM // 128      # 32 contraction chunks
NCH = 8               # (batch, qc) chunks per core
NPC = 4               # x pieces per chunk (8 kk each)
SCALE = 1.0 / float(np.sqrt(HD))

# de-interleave permutation within one head_dim: evens then odds
PERM = np.concatenate([np.arange(0, HD, 2), np.arange(1, HD, 2)])

_CACHE = {}
LAST_RESULT = None


def _emit(tc_ctx, nc, tens, phases=3, first=True):
    dt = mybir.dt
    AF = mybir.ActivationFunctionType
    xs, wd, wod, cosd, sind, trid, bqd, bkd, out = tens

    ctx = tc_ctx._emit_ctx
    cns = tc_ctx._emit_cns

    (w_res, wo_res, cos_sb, sin_sb, tri_sb, ident, ones_col, bq_sb,
     bk_sb) = cns["consts"]
    kvp = cns["kvp"]
    xring = cns["xring"]
    qring = cns["qring"]
    aring = cns["aring"]
    stage = cns["stage"]
    epool = cns["epool"]
    espool = cns["espool"]
    npool = cns["npool"]
    obpool = cns["obpool"]
    ps_ph1 = cns["ps_ph1"]
    ps_st = cns["ps_st"]
    ps_wk = cns["ps_wk"]

    k_sb = cns["k_sb"]    # [128, 2048] per batch
    v_sb = cns["v_sb"]    # [128, 16*128] per batch (block tt at cols tt*128)

    q_t = [None, None]    # chunk ring handles
    a_t = [None, None]

    def dma_x(ch):
        for p in range(NPC):
            xp = xring.tile([128, 8 * TC], dt.bfloat16, name="xp", tag="xp")
            nc.sync.dma_start(
                xp[:], xs[ch * 128:(ch + 1) * 128, p * 8 * TC:(p + 1) * 8 * TC])
            cns.setdefault("xp", {})[(ch % 2, p)] = xp

    def rope(pst, dst, bias_ap, qc):
        sl = slice(qc * TC, (qc + 1) * TC)
        pre = stage.tile([128, TC], dt.bfloat16, name="pre", tag="pre")
        nc.vector.tensor_scalar_add(pre[:], pst[:], bias_ap)
        rot = stage.tile([128, TC], dt.bfloat16, name="rot", tag="rot")
        nc.gpsimd.dma_start(rot[0:64, :], pre[64:128, :])
        nc.gpsimd.dma_start(rot[64:128, :], pre[0:64, :])
        nc.vector.tensor_mul(pre[:], pre[:], cos_sb[:, sl])
        nc.vector.tensor_mul(rot[:], rot[:], sin_sb[:, sl])
        nc.vector.tensor_add(dst[:], pre[:], rot[:])

    SGS = ((0, 1), (2, 3), (4,), (5,))  # m-tiles per subgroup: Q01 Q23 V K

    def ph1_sg(ch, sg):
        """Projection subgroup sg of chunk ch."""
        b, qc = ch // 4, ch % 4
        if sg == 0:
            q_t[ch % 2] = qring.tile([128, QH * TC], dt.bfloat16, name="qt",
                                     tag="qt")
        ms = SGS[sg]
        pss = [ps_ph1.tile([128, TC], dt.float32, name="acc", tag="acc")
               for _ in ms]
        for kk in range(NKK):
            xp = cns["xp"][(ch % 2, kk // 8)]
            rhs = xp[:, (kk % 8) * TC:(kk % 8 + 1) * TC]
            for j, m in enumerate(ms):
                nc.tensor.matmul(
                    pss[j][:], wd_sb(m, kk), rhs,
                    start=(kk == 0), stop=(kk == NKK - 1),
                    skip_group_check=True)
        for j, m in enumerate(ms):
            if m < QH:      # Q head
                rope(pss[j], q_t[ch % 2][:, m * TC:(m + 1) * TC],
                     bq_sb[:, m:m + 1], qc)
            elif m == QH:   # V
                vtmp = stage.tile([128, TC], dt.bfloat16, name="vtmp",
                                  tag="vtmp")
                nc.vector.tensor_copy(vtmp[:], pss[j][:])
                for tl in range(4):
                    pvt = ps_wk.tile([128, 128], dt.bfloat16, name="pvt",
                                     tag="wk")
                    nc.tensor.transpose(
                        pvt[:], vtmp[:, tl * 128:(tl + 1) * 128], ident[:])
                    tt = qc * 4 + tl
                    nc.scalar.copy(
                        v_sb[b][:, tt * 128:(tt + 1) * 128], pvt[:])
            else:           # K
                rope(pss[j], k_sb[b][:, qc * TC:(qc + 1) * TC],
                     bk_sb[:, 0:1], qc)

    def wd_sb(m, kk):
        return w_res[:, kk * NM * 128 + m * 128:kk * NM * 128 + (m + 1) * 128]

    def attn_head(ch, h, depth=3):
        b, qc = ch // 4, ch % 4
        nkt = 4 * qc + 4
        qt = q_t[ch % 2]

        def st_of(kt):
            o = (kt - 4 * qc) * 128 if kt >= 4 * qc else 0
            st = ps_st.tile([128, TC], dt.float32, name="st", tag="st")
            nc.tensor.matmul(
                st[:, o:TC], k_sb[b][:, kt * 128:(kt + 1) * 128],
                qt[:, h * TC + o:(h + 1) * TC], start=True, stop=True,
                skip_group_check=True)
            return st, o

        pv = ps_wk.tile([128, TC], dt.float32, name="pv", tag="wk")
        esum = espool.tile([128, TC], dt.bfloat16, name="esum", tag="esum")
        sts = [st_of(k) for k in range(min(depth, nkt))]
        for kt in range(nkt):
            if kt + depth < nkt:
                sts.append(st_of(kt + depth))
            st, o = sts[kt]
            e = epool.tile([128, TC], dt.bfloat16, name="expS", tag="e")
            nc.scalar.activation(e[:, o:TC], st[:, o:TC], AF.Exp, scale=SCALE)
            if kt >= 4 * qc:
                nc.vector.tensor_mul(e[:, o:o + 128], e[:, o:o + 128],
                                     tri_sb[:])
            if kt == 0:
                nc.vector.tensor_copy(esum[:], e[:])
            else:
                nc.vector.tensor_add(esum[:, o:TC], esum[:, o:TC], e[:, o:TC])
            nc.tensor.matmul(pv[:, o:TC],
                             v_sb[b][:, kt * 128:(kt + 1) * 128],
                             e[:, o:TC], start=(kt == 0),
                             stop=(kt == nkt - 1), skip_group_check=True)
        sums = ps_wk.tile([1, TC], dt.float32, name="sums", tag="wk")
        nc.tensor.matmul(sums[:], ones_col[:], esum[:], start=True, stop=True,
                         skip_group_check=True)
        recip = npool.tile([1, TC], dt.float32, name="recip", tag="recip")
        nc.vector.reciprocal(recip[:], sums[:])
        bcs = npool.tile([128, TC], dt.float32, name="bcs", tag="bcs")
        nc.gpsimd.partition_broadcast(bcs[:], recip[:])
        nc.vector.tensor_mul(a_t[ch % 2][:, h * TC:(h + 1) * TC], pv[:],
                             bcs[:])

    def ph3(ch):
        b, qc = ch // 4, ch % 4
        at = a_t[ch % 2]
        n = 0
        for tl in range(4):
            for oc in range(8):
                po = ps_wk.tile([128, TC], dt.float32, name="po", tag="wk")
                for h in range(QH):
                    nc.tensor.matmul(
                        po[:], at[:, h * TC + tl * 128:h * TC + tl * 128 + 128],
                        wo_res[:, h * DIM + oc * TC:h * DIM + (oc + 1) * TC],
                        start=(h == 0), stop=(h == QH - 1),
                        skip_group_check=True)
                ob = obpool.tile([128, TC], dt.float32, name="ob", tag="ob")
                if n % 2 == 0:
                    nc.vector.tensor_copy(ob[:], po[:])
                else:
                    nc.scalar.copy(ob[:], po[:])
                eng = nc.sync if n % 2 == 0 else nc.scalar
                eng.dma_start(
                    out[b * SEQ + qc * TC + tl * 128:
                        b * SEQ + qc * TC + tl * 128 + 128,
                        oc * TC:(oc + 1) * TC], ob[:])
                n += 1

    # ---------------- body ----------------
    if phases == 1:
        for ch in range(NCH):
            dma_x(ch)
            a_t[ch % 2] = aring.tile([128, QH * TC], dt.bfloat16, name="at",
                                     tag="at")
            for sg in range(4):
                ph1_sg(ch, sg)
        for b in range(2):
            nc.sync.dma_start(out[b * SEQ:b * SEQ + 128, 0:1024],
                              k_sb[b][:, :].bitcast(dt.float32))
            nc.sync.dma_start(out[b * SEQ + 128:b * SEQ + 256, 0:1024],
                              v_sb[b][:, :].bitcast(dt.float32))
        for r in range(2):
            if q_t[r] is not None:
                nc.sync.dma_start(out[r * 128:(r + 1) * 128, 1024:2048],
                                  q_t[r][:, :].bitcast(dt.float32))
        return

    dma_x(0)
    a_t[0] = aring.tile([128, QH * TC], dt.bfloat16, name="at", tag="at")
    for sg in range(4):
        ph1_sg(0, sg)
    for i in range(NCH):
        if i < NCH - 1:
            dma_x(i + 1)
            a_t[(i + 1) % 2] = aring.tile([128, QH * TC], dt.bfloat16,
                                          name="at", tag="at")
        for h in range(QH):
            attn_head(i, h)
            if i < NCH - 1:
                ph1_sg(i + 1, h)
        if phases >= 3:
            ph3(i)
    if phases == 2:
        for r in range(2):
            nc.sync.dma_start(out[r * 128:(r + 1) * 128, 2048:3072],
                              a_t[r][:, :].bitcast(dt.float32))


def build_nc(num_devices=8, reps=1, phases=3):
    nc = bacc.Bacc("TRN2", target_bir_lowering=False, debug=False,
                   enable_asserts=False, num_devices=num_devices)
    dt = mybir.dt
    xs = nc.dram_tensor("xs", [NCH * 128, NKK * TC], dt.bfloat16,
                        kind="ExternalInput").ap()
    wd = nc.dram_tensor("wT", [128, NKK * NM * 128], dt.bfloat16,
                        kind="ExternalInput").ap()
    wod = nc.dram_tensor("woT", [128, QH * DIM], dt.bfloat16,
                         kind="ExternalInput").ap()
    cosd = nc.dram_tensor("cos128", [HD, SEQ], dt.bfloat16,
                          kind="ExternalInput").ap()
    sind = nc.dram_tensor("sin128s", [HD, SEQ], dt.bfloat16,
                          kind="ExternalInput").ap()
    trid = nc.dram_tensor("tri", [128, 128], dt.bfloat16,
                          kind="ExternalInput").ap()
    bqd = nc.dram_tensor("bq_sb", [HD, QH], dt.float32,
                         kind="ExternalInput").ap()
    bkd = nc.dram_tensor("bk_sb", [HD, 1], dt.float32,
                         kind="ExternalInput").ap()
    out = nc.dram_tensor("out", [2 * SEQ, DIM], dt.float32,
                         kind="ExternalOutput").ap()
    tens = (xs, wd, wod, cosd, sind, trid, bqd, bkd, out)

    with tile.TileContext(nc) as tctx:
        with ExitStack() as ctx:
            cns = {}
            tctx._emit_ctx = ctx
            tctx._emit_cns = cns
            cpool = ctx.enter_context(tc_pool(tctx, "consts", 1))
            w_res = cpool.tile([128, NKK * NM * 128], dt.bfloat16, name="w_res")
            wo_res = cpool.tile([128, QH * DIM], dt.bfloat16, name="wo_res")
            cos_sb = cpool.tile([HD, SEQ], dt.bfloat16, name="cos_sb")
            sin_sb = cpool.tile([HD, SEQ], dt.bfloat16, name="sin_sb")
            tri_sb = cpool.tile([128, 128], dt.bfloat16, name="tri_sb")
            ident = cpool.tile([128, 128], dt.bfloat16, name="ident")
            ones_col = cpool.tile([128, 1], dt.bfloat16, name="ones_col")
            bq_sb = cpool.tile([HD, QH], dt.float32, name="bq_sb")
            bk_sb = cpool.tile([HD, 1], dt.float32, name="bk_sb")
            cns["consts"] = (w_res, wo_res, cos_sb, sin_sb, tri_sb, ident,
                             ones_col, bq_sb, bk_sb)
            kvpool = ctx.enter_context(tc_pool(tctx, "kvp", 1))
            cns["kvp"] = kvpool
            cns["k_sb"] = [kvpool.tile([128, SEQ], dt.bfloat16, name=f"k{b}")
                           for b in range(2)]
            cns["v_sb"] = [kvpool.tile([128, SEQ], dt.bfloat16, name=f"v{b}")
                           for b in range(2)]
            cns["xring"] = ctx.enter_context(tc_pool(tctx, "xr", 6))
            cns["qring"] = ctx.enter_context(tc_pool(tctx, "qr", 2))
            cns["aring"] = ctx.enter_context(tc_pool(tctx, "ar", 2))
            cns["stage"] = ctx.enter_context(tc_pool(tctx, "stg", 2))
            cns["epool"] = ctx.enter_context(tc_pool(tctx, "ep", 8))
            cns["espool"] = ctx.enter_context(tc_pool(tctx, "esp", 2))
            cns["npool"] = ctx.enter_context(tc_pool(tctx, "np", 2))
            cns["obpool"] = ctx.enter_context(tc_pool(tctx, "obp", 4))
            cns["ps_ph1"] = ctx.enter_context(
                tc_pool(tctx, "psph1", 2, space="PSUM"))
            cns["ps_st"] = ctx.enter_context(
                tc_pool(tctx, "psst", 3, space="PSUM"))
            cns["ps_wk"] = ctx.enter_context(
                tc_pool(tctx, "pswk", 3, space="PSUM"))

            # one-time constant loads (outside the reps loop)
            nc.scalar.dma_start(w_res[:, 0:NKK * NM * 32],
                                wd[:, 0:NKK * NM * 32])
            nc.scalar.dma_start(w_res[:, NKK * NM * 32:NKK * NM * 64],
                                wd[:, NKK * NM * 32:NKK * NM * 64])
            nc.scalar.dma_start(w_res[:, NKK * NM * 64:NKK * NM * 96],
                                wd[:, NKK * NM * 64:NKK * NM * 96])
            nc.scalar.dma_start(w_res[:, NKK * NM * 96:NKK * NM * 128],
                                wd[:, NKK * NM * 96:NKK * NM * 128])
            nc.scalar.dma_start(wo_res[:], wod[:])
            nc.scalar.dma_start(cos_sb[:], cosd[:])
            nc.scalar.dma_start(sin_sb[:], sind[:])
            nc.scalar.dma_start(tri_sb[:], trid[:])
            nc.scalar.dma_start(bq_sb[:], bqd[:])
            nc.scalar.dma_start(bk_sb[:], bkd[:])
            nc.vector.memset(ones_col[:], 1.0)
            masks_mod.make_identity(nc, ident[:])

            if reps == 1:
                _emit(tctx, nc, tens, phases=phases)
            else:
                with tctx.For_i(0, reps, 1):
                    _emit(tctx, nc, tens, phases=phases)
    nc.compile()
    return nc


def tc_pool(tctx, name, bufs, space="SBUF"):
    return tctx.tile_pool(name=name, bufs=bufs, space=space)


def _get_nc():
    if "nc" not in _CACHE:
        _CACHE["nc"] = build_nc()
    return _CACHE["nc"]


def make_in_maps(x, freqs_cos, freqs_sin, mask, wq, bq, wk, bk, wv, bv,
                 wo, bo=None):
    x = np.asarray(x, F32)
    # xs: [8 chunks, 128, 32 kk * 512] with 32KB contiguous rows
    X = np.zeros((NCH, 128, NKK * TC), F32)
    for b in range(2):
        xT = np.ascontiguousarray(x[b].T)  # [4096, 2048]
        blk = xT.reshape(NKK, 128, 4, TC).transpose(2, 1, 0, 3)  # [qc,128,kk,TC]
        X[b * 4:(b + 1) * 4] = blk.reshape(4, 128, NKK * TC)
    xs = np.ascontiguousarray(X.reshape(NCH * 128, NKK * TC)).astype(BF16)

    cosT = np.asarray(freqs_cos, F32).T  # [64, 2048]
    sinT = np.asarray(freqs_sin, F32).T
    cos128 = np.ascontiguousarray(np.vstack([cosT, cosT])).astype(BF16)
    sin128s = np.ascontiguousarray(np.vstack([-sinT, sinT])).astype(BF16)
    mask = np.asarray(mask, F32)
    tri = np.ascontiguousarray((mask[0:128, 0:128] == 0).T).astype(BF16)

    wq, wk, wv, wo = (np.asarray(a, F32) for a in (wq, wk, wv, wo))
    bq, bk = np.asarray(bq, F32), np.asarray(bk, F32)
    in_maps = []
    for c in range(8):
        qrows = wq[c * 512:(c + 1) * 512].reshape(QH, HD, DIM)[:, PERM, :]
        vrows = wv[c * 128:(c + 1) * 128]
        krows = wk[c * 128:(c + 1) * 128][PERM, :]
        wcat = np.concatenate([qrows.reshape(QH * HD, DIM), vrows, krows],
                              axis=0)  # [768, 4096] m-order Q0..Q3 V K
        wT_c = wcat.T  # [4096, 768]
        W = np.ascontiguousarray(
            wT_c.reshape(NKK, 128, NM * 128).transpose(1, 0, 2)
            .reshape(128, NKK * NM * 128)).astype(BF16)
        woT_c = wo[:, c * 512:(c + 1) * 512].T  # [512, 4096]
        WO = np.ascontiguousarray(
            woT_c.reshape(QH, 128, DIM).transpose(1, 0, 2)
            .reshape(128, QH * DIM)).astype(BF16)
        bq_c = np.ascontiguousarray(
            bq[c * 512:(c + 1) * 512].reshape(QH, HD)[:, PERM].T).astype(F32)
        bk_c = np.ascontiguousarray(
            bk[c * 128:(c + 1) * 128][PERM].reshape(HD, 1)).astype(F32)
        in_maps.append(dict(xs=xs, wT=W, woT=WO, cos128=cos128,
                            sin128s=sin128s, tri=tri, bq_sb=bq_c, bk_sb=bk_c))
    return in_maps


def kernel(x, freqs_cos, freqs_sin, mask, wq, bq, wk, bk, wv, bv, wo, bo):
    global LAST_RESULT
    nc = _get_nc()
    in_maps = make_in_maps(x, freqs_cos, freqs_sin, mask, wq, bq, wk, bk,
                           wv, bv, wo, bo)
    res = run_bass_kernel_spmd(nc, in_maps, core_ids=list(range(8)))
    LAST_RESULT = res
    outp = np.zeros((2, SEQ, DIM), F32)
    for c in range(8):
        part = np.asarray(res.results[c]["out"], F32)
        outp[0] += part[0:SEQ]
        outp[1] += part[SEQ:2 * SEQ]
    bv = np.asarray(bv, F32)
    bo = np.asarray(bo, F32)
    wo = np.asarray(wo, F32)
    bv_exp = np.broadcast_to(
        bv.reshape(N_KV, 1, HD), (N_KV, N_HEADS // N_KV, HD)).reshape(DIM)
    outp += (bv_exp @ wo.T + bo)[None, None, :].astype(F32)
    return outp
